# revision 1
# baseline (speedup 1.0000x reference)
"""Trainium2 Bass kernel for MHA (B=2, S=2048, D=512, H=8, dk=dv=32) + additive mask.

Sharding: core c -> batch c//4, query slice (c%4)*512. Scores are computed
transposed ([k, q]) so softmax sums ride the PE (ones-matmul) and the AV
contraction has keys on partitions.

v2 structure (vs the v1 baseline):
- Attention runs in 2 head-group passes (heads 0-3 / 4-7) over 16 key chunks
  x 2 query halves; the 4 QK matmuls of a slot are row-tiled 4-way
  (tile_position 32j) so they stream concurrently.
- exp(scores) is split across engines: most slots use ACT (exp(s)*exp(m),
  mask exp'd on host), a configurable subset uses a fused one-op DVE
  Schraudolph approximation (bits_i16 = s*A + (m*A + B), bitcast to bf16),
  and part of the EM multiplies run on GpSimd.
- K/V projections are computed lazily inside pass A's kc loop to keep the
  PE dense (HAM stays at 2.4 GHz).
- Softmax denominators accumulate via ones-matmuls into one PSUM bank per
  pass; one reciprocal_approx_fast + one SEL-matmul replicates 1/l across
  partitions (v1 spent 32us in single-partition reciprocals here).
"""

import numpy as np
import ml_dtypes

B, S, D, DK, H, DH = 2, 2048, 512, 256, 8, 32
QR = 512
NCORES = 8
BF = ml_dtypes.bfloat16
F16h = np.float16

A16 = 184.663956  # 2^7 / ln2
C_CORR = 6.0      # Schraudolph bias correction (bf16-bit units)
B16 = 127.0 * 128.0 - C_CORR

# slot assignment: (kc, qh) -> engine for the exp/mult stage
import os as _os
_FEAT = _os.environ.get("KFEAT", "zgDOMR")
Z_SLOTS = ({(kc, p, pr) for kc in range(9, 16, 2) for p in range(2)
            for pr in range(2)} if "z" in _FEAT else set())
GPS_MULT = ({(kc, p, 0) for kc in range(2, 16, 4) for p in range(2)}
            if "g" in _FEAT else set())
F_CHUNK_DMA = "D" in _FEAT
F_F16_OUT = "O" in _FEAT
F_MEMSET = "M" in _FEAT
F_RECIP_APPROX = "R" in _FEAT

_CACHED = {}


def _body(nc, tc, mybir, aps):
    bf16 = mybir.dt.bfloat16
    f32 = mybir.dt.float32
    f16 = mybir.dt.float16
    i16 = mybir.dt.int16
    Exp = mybir.ActivationFunctionType.Exp
    Ident = mybir.ActivationFunctionType.Identity
    Alu = mybir.AluOpType
    (qt, kt, vt, em, m2, wq, wk, wv, wo, bqk, bos, sel, out) = aps

    with (
        tc.tile_pool(name="cst", bufs=1) as cp,
        tc.tile_pool(name="p1p", bufs=3) as p1p,
        tc.tile_pool(name="p2p", bufs=3) as p2p,
        tc.tile_pool(name="qkp", bufs=2, space="PSUM") as qkp,
        tc.tile_pool(name="accp", bufs=1, space="PSUM") as accp,
    ):
        # ---- persistent SBUF ----
        QT = cp.tile([128, 4 * QR], bf16, tag="QT")
        KT = cp.tile([128, 4 * S], bf16, tag="KT")       # [Dc][128, k]
        VT = cp.tile([128, 4 * S], bf16, tag="VT")
        EM = cp.tile([128, 16 * QR], bf16, tag="EM")     # [kc][128k, 512q]
        M2 = cp.tile([128, 4 * 512], f32, tag="M2")      # kc 9,11,13,15
        WQ = cp.tile([128, 4 * DK], bf16, tag="WQ")
        WK = cp.tile([128, 4 * DK], bf16, tag="WK")
        WV = cp.tile([128, 4 * DK], bf16, tag="WV")
        WO = cp.tile([128, 4 * D], bf16, tag="WO")
        BQK = cp.tile([128, 4], f32, tag="BQK")
        BOS = cp.tile([1, D], bf16, tag="BOS")
        SEL = cp.tile([128, 128], bf16, tag="SEL")

        # loads ordered by first use; big tensors chunked so early kc
        # iterations start before the tail lands
        nc.sync.dma_start(out=WQ, in_=wq)
        nc.sync.dma_start(out=QT, in_=qt)
        nc.sync.dma_start(out=WK, in_=wk)
        nc.sync.dma_start(out=WV, in_=wv)
        nc.sync.dma_start(out=BQK, in_=bqk)
        if F_CHUNK_DMA:
            for g in range(4):
                nc.sync.dma_start(out=KT.rearrange("p (dc k) -> p dc k", dc=4)
                                  [:, :, g * 512:(g + 1) * 512],
                                  in_=kt.rearrange("p (dc k) -> p dc k", dc=4)
                                  [:, :, g * 512:(g + 1) * 512])
                nc.sync.dma_start(out=VT.rearrange("p (dc k) -> p dc k", dc=4)
                                  [:, :, g * 512:(g + 1) * 512],
                                  in_=vt.rearrange("p (dc k) -> p dc k", dc=4)
                                  [:, :, g * 512:(g + 1) * 512])
                nc.sync.dma_start(out=EM[:, g * 4 * QR:(g + 1) * 4 * QR],
                                  in_=em[:, g * 4 * QR:(g + 1) * 4 * QR])
        else:
            nc.sync.dma_start(out=KT, in_=kt)
            nc.sync.dma_start(out=VT, in_=vt)
            nc.sync.dma_start(out=EM, in_=em)
        nc.sync.dma_start(out=SEL, in_=sel)
        nc.sync.dma_start(out=WO, in_=wo)
        nc.sync.dma_start(out=BOS, in_=bos)
        for g in range(2):
            nc.sync.dma_start(out=M2[:, g * 1024:(g + 1) * 1024],
                              in_=m2[:, g * 1024:(g + 1) * 1024])
        ONES = cp.tile([128, 1], bf16, tag="ONES")
        nc.vector.memset(ONES, 1.0)
        ONE_ROW = cp.tile([1, 128], bf16, tag="ONE_ROW")
        nc.vector.memset(ONE_ROW, 1.0)

        qT = [cp.tile([128, QR], bf16, tag=f"qT{d}", name=f"qT{d}")
              for d in range(2)]
        kT = [cp.tile([128, S], bf16, tag=f"kT{d}", name=f"kT{d}")
              for d in range(2)]
        v_sb = cp.tile([128, 16 * 512], bf16, tag="v_sb")
        # per (kc, head): 64-col block [v(32) | ones(1) | zeros(31)] so one
        # M=64 matmul accumulates both context and the softmax denominator
        nc.vector.memset(v_sb, 0.0)
        nc.vector.memset(
            v_sb.rearrange("p (kc h c) -> p kc h c", h=8, c=64)[:, :, :, 32:33],
            1.0)
        ctx = cp.tile([128, 2 * QR], bf16, tag="ctx")     # [dvc][128dv, 512q]
        r_sb = cp.tile([128, QR], f32, tag="r_sb")
        rb16 = cp.tile([128, QR], bf16, tag="rb16")
        rq_sb = cp.tile([128, QR], bf16, tag="rq_sb")
        out_sb = cp.tile([128, 4 * D], f16 if F_F16_OUT else f32,
                         tag="out_sb")

        # ---- Q projection ----
        for dkc in range(2):
            ps = qkp.tile([128, QR], f32, tag="qk")
            for Dc in range(4):
                nc.tensor.matmul(
                    ps,
                    lhsT=WQ[:, Dc * DK + dkc * 128:Dc * DK + dkc * 128 + 128],
                    rhs=QT[:, Dc * QR:(Dc + 1) * QR],
                    start=(Dc == 0), stop=(Dc == 3))
            nc.vector.tensor_scalar_add(qT[dkc], ps, BQK[:, dkc:dkc + 1])

        def project_kv(kcg):
            # kT for keys kcg*512 .. +512 (both dkc halves) + v for 4 kcs
            for dkc in range(2):
                ps = qkp.tile([128, 512], f32, tag="qk")
                for Dc in range(4):
                    nc.tensor.matmul(
                        ps,
                        lhsT=WK[:, Dc * DK + dkc * 128:Dc * DK + dkc * 128 + 128],
                        rhs=KT[:, Dc * S + kcg * 512:Dc * S + kcg * 512 + 512],
                        start=(Dc == 0), stop=(Dc == 3))
                nc.vector.tensor_scalar_add(
                    kT[dkc][:, kcg * 512:(kcg + 1) * 512], ps,
                    BQK[:, 2 + dkc:3 + dkc])
            for kk in range(4):
                kc = kcg * 4 + kk
                ps = qkp.tile([128, DK], f32, tag="qk")
                for Dc in range(4):
                    nc.tensor.matmul(
                        ps,
                        lhsT=VT[:, Dc * S + kc * 128:Dc * S + kc * 128 + 128],
                        rhs=WV[:, Dc * DK:(Dc + 1) * DK],
                        start=(Dc == 0), stop=(Dc == 3))
                nc.vector.tensor_copy(
                    v_sb.rearrange("p (kc h c) -> p kc h c", h=8, c=64)
                    [:, kc, :, 0:32],
                    ps.rearrange("p (h c) -> p h c", c=32))

        # ---- attention: both head-group passes interleaved per kc ----
        # PSUM rule: every matmul output owns a full 512-col f32 bank row
        # (256-col sub-bank writes hang the device). Interleaving the two
        # passes keeps the PE densely busy so the HAM clock stays at 2.4GHz.
        avb = {}
        for p in range(2):
            for b in range(2):
                avb[p, b] = accp.tile([128, QR], f32, tag=f"av{p}{b}",
                                      name=f"av{p}{b}")
        for kc in range(16):
            if kc % 4 == 0:
                project_kv(kc // 4)
            st, sp_ = (kc == 0), (kc == 15)
            for p in range(2):
                dkc = p
                p2s = []
                for pr in range(2):
                    qk = qkp.tile([128, 1024], f32, tag="qk")
                    for jj in range(2):
                        j = 2 * pr + jj
                        nc.tensor.matmul(
                            qk[:, jj * 512:(jj + 1) * 512],
                            lhsT=kT[dkc][32 * j:32 * j + 32,
                                         kc * 128:kc * 128 + 128],
                            rhs=qT[dkc][32 * j:32 * j + 32, :],
                            start=True, stop=True, tile_position=(32 * j, 0))
                    p2 = p2p.tile([128, 1024], bf16, tag="p2")
                    if (kc, p, pr) in Z_SLOTS:
                        zi = (kc - 9) // 2
                        m2b = M2[:, zi * 512:(zi + 1) * 512].rearrange(
                            "p (a b) -> p a b", a=1).broadcast_to((128, 2, 512))
                        nc.vector.scalar_tensor_tensor(
                            out=p2.bitcast(i16).rearrange(
                                "p (a b) -> p a b", b=512),
                            in0=qk.rearrange("p (a b) -> p a b", b=512),
                            scalar=A16, in1=m2b, op0=Alu.mult, op1=Alu.add)
                    else:
                        p1 = p1p.tile([128, 1024], bf16, tag="p1")
                        nc.scalar.activation(p1, qk, Exp)
                        emb = EM[:, kc * QR:(kc + 1) * QR].rearrange(
                            "p (a b) -> p a b", a=1).broadcast_to((128, 2, 512))
                        eng = (nc.gpsimd if (kc, p, pr) in GPS_MULT
                               else nc.vector)
                        eng.tensor_tensor(
                            out=p2.rearrange("p (a b) -> p a b", b=512),
                            in0=p1.rearrange("p (a b) -> p a b", b=512),
                            in1=emb, op=Alu.mult)
                    p2s.append(p2)
                for j in range(4):
                    h = 4 * p + j
                    nc.tensor.matmul(
                        avb[p, j // 2][64 * (j % 2):64 * (j % 2) + 64, :],
                        lhsT=v_sb[:, kc * 512 + 64 * h:kc * 512 + 64 * h + 64],
                        rhs=p2s[j // 2][:, (j % 2) * 512:(j % 2) * 512 + 512],
                        start=st, stop=sp_, tile_position=(0, 64 * (j % 2)),
                        skip_group_check=True)
        # ---- finalize: ctx_aug = av / l (rows 32/96 of each bank hold l) ----
        ctxa = cp.tile([128, 4 * QR], bf16, tag="ctxa")
        for p in range(2):
            for b in range(2):
                av = avb[p, b]
                if F_RECIP_APPROX:
                    nc.vector.reciprocal_approx_fast(out=r_sb, in_=av)
                else:
                    nc.vector.reciprocal(r_sb, av)
                nc.vector.tensor_scalar(out=rb16, in0=r_sb, scalar1=0.0,
                                        scalar2=3e38, op0=Alu.max, op1=Alu.min)
                rq = qkp.tile([128, QR], f32, tag="qk")
                nc.tensor.matmul(rq, lhsT=SEL, rhs=rb16, start=True, stop=True)
                nc.scalar.copy(rq_sb, rq)
                nc.vector.tensor_tensor(
                    out=ctxa[:, (2 * p + b) * QR:(2 * p + b + 1) * QR],
                    in0=av, in1=rq_sb, op=Alu.mult)

        # ---- output projection (wo_aug has zero rows at l/junk slots) ----
        for qc in range(4):
            ps = qkp.tile([128, D], f32, tag="qk")
            for pb in range(4):
                nc.tensor.matmul(
                    ps,
                    lhsT=ctxa[:, pb * QR + qc * 128:pb * QR + qc * 128 + 128],
                    rhs=WO[:, pb * D:(pb + 1) * D],
                    start=(pb == 0), stop=False)
            nc.tensor.matmul(ps, lhsT=ONE_ROW, rhs=BOS,
                             start=False, stop=True)
            nc.scalar.copy(out_sb[:, qc * D:(qc + 1) * D], ps)
        nc.sync.dma_start(
            out=out.rearrange("(qc p) d -> p qc d", p=128),
            in_=out_sb.rearrange("p (qc d) -> p qc d", d=D))


def _build():
    if "nc" in _CACHED:
        return _CACHED["nc"]
    import concourse.bacc as bacc
    import concourse.tile as tile
    import concourse.mybir as mybir

    bf16 = mybir.dt.bfloat16
    f32 = mybir.dt.float32
    f16 = mybir.dt.float16
    nc = bacc.Bacc("TRN2", target_bir_lowering=False, debug=False,
                   enable_asserts=False, num_devices=NCORES)
    aps = [
        nc.dram_tensor("qt", [128, 4 * QR], bf16, kind="ExternalInput").ap(),
        nc.dram_tensor("kt", [128, 4 * S], bf16, kind="ExternalInput").ap(),
        nc.dram_tensor("vt", [128, 4 * S], bf16, kind="ExternalInput").ap(),
        nc.dram_tensor("em", [128, 16 * QR], bf16, kind="ExternalInput").ap(),
        nc.dram_tensor("m2", [128, 4 * 512], f32, kind="ExternalInput").ap(),
        nc.dram_tensor("wq", [128, 4 * DK], bf16, kind="ExternalInput").ap(),
        nc.dram_tensor("wk", [128, 4 * DK], bf16, kind="ExternalInput").ap(),
        nc.dram_tensor("wv", [128, 4 * DK], bf16, kind="ExternalInput").ap(),
        nc.dram_tensor("wo", [128, 4 * D], bf16, kind="ExternalInput").ap(),
        nc.dram_tensor("bqk", [128, 4], f32, kind="ExternalInput").ap(),
        nc.dram_tensor("bos", [1, D], bf16, kind="ExternalInput").ap(),
        nc.dram_tensor("sel", [128, 128], bf16, kind="ExternalInput").ap(),
        nc.dram_tensor("out", [QR, D],
                       f16 if F_F16_OUT else f32, kind="ExternalOutput").ap(),
    ]
    with tile.TileContext(nc) as tc:
        _body(nc, tc, mybir, aps)
    nc.compile()
    _CACHED["nc"] = nc
    return nc


def _block4(x):
    c = x.shape[1]
    return np.ascontiguousarray(
        x.reshape(4, 128, c).transpose(1, 0, 2).reshape(128, 4 * c))


def make_in_maps(V, Q, K, mask, Wq, bq, Wk, bk, Wv, bv, Wo, bo):
    f = np.float32
    V, Q, K, mask = (np.asarray(a, f) for a in (V, Q, K, mask))
    Wq, bq, Wk, bk, Wv, bv, Wo, bo = (
        np.asarray(a, f) for a in (Wq, bq, Wk, bk, Wv, bv, Wo, bo))
    denom = np.sqrt(f(DK))
    wq_h = _block4(Wq / denom).astype(BF)
    wk_h = _block4(Wk).astype(BF)
    wv_h = _block4(Wv).astype(BF)
    # wo_aug[pb]: rows 0-31 = Wo rows of head 2*pb, 64-95 = head 2*pb+1,
    # zeros at the l/junk row slots
    wo_h = np.zeros((128, 4 * D), np.float32)
    for pb in range(4):
        wo_h[0:32, pb * D:(pb + 1) * D] = Wo[(2 * pb) * 32:(2 * pb) * 32 + 32]
        wo_h[64:96, pb * D:(pb + 1) * D] = \
            Wo[(2 * pb + 1) * 32:(2 * pb + 1) * 32 + 32]
    wo_h = np.ascontiguousarray(wo_h).astype(BF)
    bqk_h = np.ascontiguousarray(
        np.concatenate([(bq / denom).reshape(2, 128).T,
                        bk.reshape(2, 128).T], axis=1)).astype(f)
    bos_h = np.ascontiguousarray((bv @ Wo + bo).reshape(1, D)).astype(BF)
    sel_h = np.zeros((128, 128), f)
    sel_h[32, 0:64] = 1.0
    sel_h[96, 64:128] = 1.0
    sel_h = sel_h.astype(BF)

    in_maps = []
    for c in range(NCORES):
        b = c // 4
        qs = slice((c % 4) * QR, (c % 4 + 1) * QR)
        QT = np.ascontiguousarray(Q[b, qs, :].T)        # [512 D, 512 q]
        KT = np.ascontiguousarray(K[b].T)               # [512 D, 2048 k]
        VT = np.ascontiguousarray(V[b].T)
        MT = np.ascontiguousarray(mask[b, 0, qs, :].T)  # [2048 k, 512 q]
        em_h = np.ascontiguousarray(
            np.exp(MT).reshape(16, 128, QR).transpose(1, 0, 2)
            .reshape(128, 16 * QR)).astype(BF)
        # m2 for kc in {9,11,13,15} (full q): [128, 4*512] f32
        m2_blk = MT.reshape(16, 128, QR)[9::2]  # [4, 128, 512]
        m2_h = np.ascontiguousarray(
            (m2_blk * A16 + B16).transpose(1, 0, 2).reshape(128, 4 * QR)
        ).astype(f)
        in_maps.append({
            "qt": _block4(QT).astype(BF),
            "kt": _block4(KT).astype(BF),
            "vt": _block4(VT).astype(BF),
            "em": em_h, "m2": m2_h,
            "wq": wq_h, "wk": wk_h, "wv": wv_h, "wo": wo_h,
            "bqk": bqk_h, "bos": bos_h, "sel": sel_h,
        })
    return in_maps


def kernel(V, Q, K, mask, Wq, bq, Wk, bk, Wv, bv, Wo, bo):
    from concourse.bass_utils import run_bass_kernel_spmd
    nc = _build()
    in_maps = make_in_maps(V, Q, K, mask, Wq, bq, Wk, bk, Wv, bv, Wo, bo)
    res = run_bass_kernel_spmd(nc, in_maps, core_ids=list(range(NCORES)))
    out_full = np.empty((B, S, D), np.float32)
    for c in range(NCORES):
        out_full[c // 4, (c % 4) * QR:(c % 4 + 1) * QR, :] = \
            res.results[c]["out"].astype(np.float32)
    return out_full

